# revision 1
# baseline (speedup 1.0000x reference)
"""AEV (ANI-1x) computer on 8 Trainium2 NeuronCores.

Data-parallel over molecules: each core computes 4 of the 32 molecules.
The angular term uses on-device neighbor-list compaction: per-center
one-hot selection matrices gather neighbor data via PE matmuls, constant
one-hot matrices expand neighbors to the 55 static n1<n2 pair slots, and
per-center species-pair one-hots reduce slot features into the 10 bins.
"""
import sys
import numpy as np

sys.path.insert(0, "/opt/trn_rl_repo")

from concourse import bacc, bass, mybir, tile  # noqa: E402
from concourse.tile import add_dep_helper  # noqa: E402
from concourse.bass_utils import run_bass_kernel_spmd  # noqa: E402

# ---- problem constants ----
M, A = 32, 40
N_CORES = 8
M4 = M // N_CORES           # molecules per core
NMI = M4 * A                # (m, i) centers per core = 160
RCR, RCA = 5.2, 3.5
ETA_R, ETA_A, ZETA = 16.0, 8.0, 32.0
SHF_R = np.linspace(0.9, RCR, 17, dtype=np.float32)[:-1]    # 16
SHF_A = np.array([0.9, 1.55, 2.2, 2.85], dtype=np.float32)  # 4
SHF_Z = (np.pi / 16.0) * (2.0 * np.arange(8, dtype=np.float32) + 1.0)
NMAX = 11                   # max angular neighbors supported (data max = 10)
NPAD = 16                   # padded neighbor slots for the gather matmul
T = NMAX * (NMAX - 1) // 2  # 55 pair-slots per center
N1 = np.array([a for a in range(NMAX) for b in range(a + 1, NMAX)])
N2 = np.array([b for a in range(NMAX) for b in range(a + 1, NMAX)])
NARR = 8                    # gathered arrays: ux,uy,uz,d,spec,valid,0,0
QP = NMI // 2               # center-pairs in packed slot layout = 80
LN2 = float(np.log(2.0))
FP32 = mybir.dt.float32
BF16 = mybir.dt.bfloat16
FP16 = mybir.dt.float16
Alu = mybir.AluOpType
Act = mybir.ActivationFunctionType

CST_LAYOUT_A = [("xj", M4 * 3), ("xi", NMI * 3), ("ohm", M4 * 4),
                ("sj", M4), ("lt", A), ("neq", NMI), ("io16", NPAD),
                ("rhoR", 16), ("e1", T), ("e2", T)]
CST_LAYOUT_F = [("io10", 10), ("rho2", 4)]
CST_A_COLS = sum(w for _, w in CST_LAYOUT_A)
CST_COLS = CST_A_COLS + sum(w for _, w in CST_LAYOUT_F)

_NC_CACHE = {}


def build_nc():
    if "nc" in _NC_CACHE:
        return _NC_CACHE["nc"]
    from contextlib import ExitStack
    nc = bacc.Bacc()
    cst_e = nc.declare_dram_parameter("cst", [128, CST_COLS], FP32,
                                      isOutput=False)
    out_e = nc.declare_dram_parameter("out", [M4, A, 384], FP32, isOutput=True)

    with tile.TileContext(nc) as tc, ExitStack() as es:
        pool = es.enter_context(tc.tile_pool(name="sb", bufs=1))
        psum = es.enter_context(tc.tile_pool(name="ps", bufs=1, space="PSUM"))
        v = nc.vector
        sc = nc.scalar

        # register activation bias constants
        for cval in (1e-12, float(np.pi / 2), LN2, 1e-30, 1e-35):
            cpk = (FP32, cval)
            if cpk not in nc.const_aps.aps:
                ct = pool.tile([128, 1], FP32, name=f"cst{len(nc.const_aps.aps)}")
                v.memset(ct[:, :], cval)
                nc.const_aps.aps[cpk] = ct

        # ---- load the input/constant blob: two DMAs ----
        cst = pool.tile([128, CST_COLS], FP32)
        HC = CST_A_COLS // 2
        nc.sync.dma_start(out=cst[:A, :HC], in_=cst_e[:A, :HC])
        nc.scalar.dma_start(out=cst[:A, HC:CST_A_COLS],
                            in_=cst_e[:A, HC:CST_A_COLS])
        nc.gpsimd.dma_start(out=cst[:, CST_A_COLS:], in_=cst_e[:, CST_A_COLS:])
        off = {}
        o = 0
        for nm, wd in CST_LAYOUT_A + CST_LAYOUT_F:
            off[nm] = o
            o += wd

        def cv(nm, rows, wd):
            return cst[0:rows, off[nm]:off[nm] + wd]

        xj = cv("xj", A, M4 * 3)
        xi = cv("xi", A, NMI * 3)
        ohm = cv("ohm", A, M4 * 4)
        sj = cv("sj", A, M4)
        lt = cv("lt", A, A)
        neq = cv("neq", A, NMI)
        io16 = cv("io16", A, NPAD)
        rhoR = cv("rhoR", A, 16)
        e1c = cv("e1", NPAD, T)
        e2c = cv("e2", NPAD, T)
        io10 = cv("io10", 128, 10)
        rho2 = cv("rho2", 128, 4)

        # ================= pair stage: [40 j, 160 (m,i)] =================
        D8 = pool.tile([A, NMI * NARR], FP16)   # gather-matmul rhs
        # only arr slots 6,7 are never written; zero just those
        d8pad = bass.AP(tensor=D8.tensor, offset=D8[:, :].offset + 6,
                        ap=[D8[:, :].ap[0], [NARR, NMI], [1, 2]])
        v.memset(d8pad, 0.0)

        def d8slot(k):
            return D8[:, :].rearrange("p (mi a) -> p mi a", a=NARR)[:, :, k]

        dx = [pool.tile([A, NMI], FP32, name=f"dx{c}", tag=f"dx{c}")
              for c in range(3)]
        for c in range(3):
            in_j = bass.AP(tensor=xj.tensor, offset=xj.offset + c,
                           ap=[xj.ap[0], [3, M4], [0, A]])
            in_i = bass.AP(tensor=xi.tensor, offset=xi.offset + c,
                           ap=[xi.ap[0], [3 * A, M4], [3, A]])
            v.tensor_tensor(dx[c][:, :].rearrange("p (m i) -> p m i", m=M4),
                            in_j, in_i, op=Alu.subtract)
        dsq = pool.tile([A, NMI], FP32)
        t0 = pool.tile([A, NMI], FP32, tag="t0")
        t1 = pool.tile([A, NMI], FP32, tag="t1")
        v.tensor_tensor(t0[:, :], dx[0][:, :], dx[0][:, :], op=Alu.mult)
        v.tensor_tensor(t1[:, :], dx[1][:, :], dx[1][:, :], op=Alu.mult)
        v.tensor_tensor(t0[:, :], t0[:, :], t1[:, :], op=Alu.add)
        v.tensor_tensor(t1[:, :], dx[2][:, :], dx[2][:, :], op=Alu.mult)
        v.tensor_tensor(dsq[:, :], t0[:, :], t1[:, :], op=Alu.add)
        # masks
        maskA = pool.tile([A, NMI], FP32)
        maskR = pool.tile([A, NMI], FP32)
        v.tensor_scalar(t0[:, :], dsq[:, :], RCA * RCA, None, op0=Alu.is_lt)
        v.tensor_tensor(maskA[:, :], t0[:, :], neq, op=Alu.mult)
        v.tensor_scalar(t1[:, :], dsq[:, :], RCR * RCR, None, op0=Alu.is_lt)
        v.tensor_tensor(maskR[:, :], t1[:, :], neq, op=Alu.mult)
        v.tensor_copy(d8slot(5), maskA[:, :])
        # d (sqrt set), 1/d, unit vectors
        dpair = pool.tile([A, NMI], FP32)
        i_sqrt = sc.activation(dpair[:, :], dsq[:, :], Act.Sqrt, bias=1e-12,
                               scale=1.0)
        inv = pool.tile([A, NMI], FP32)
        v.reciprocal(inv[:, :], dpair[:, :])
        for c in range(3):
            v.tensor_tensor(d8slot(c), dx[c][:, :], inv[:, :], op=Alu.mult)
        v.tensor_copy(d8slot(3), dpair[:, :])
        # species of j replicated along i
        in_s = bass.AP(tensor=sj.tensor, offset=sj.offset,
                       ap=[sj.ap[0], [1, M4], [0, A]])
        v.tensor_copy(d8slot(4).rearrange("p (m i) -> p m i", m=M4), in_s)

        # ================= neighbor ranks via PE =================
        ps_rank = psum.tile([A, NMI], FP32, tag="ps", bufs=2)
        nc.tensor.matmul(ps_rank[:, :], lt, maskA[:, :], start=True, stop=True)
        rankp = pool.tile([A, NMI], FP32)
        # valid j -> rank (0..10); invalid -> rank - 1000
        v.scalar_tensor_tensor(rankp[:, :], maskA[:, :], 1000.0, ps_rank[:, :],
                               op0=Alu.mult, op1=Alu.add)
        v.tensor_scalar(rankp[:, :], rankp[:, :], 1000.0, None,
                        op0=Alu.subtract)
        # one-hot N [40, (mi, n)] fp16 (exact 0/1; cheap 1-pass matmuls)
        Nsel = pool.tile([A, NMI * NPAD], FP16)
        v.tensor_tensor(
            Nsel[:, :].rearrange("p (mi n) -> p mi n", n=NPAD),
            rankp[:, :].unsqueeze(2).broadcast_to((A, NMI, NPAD)),
            io16.unsqueeze(1).broadcast_to((A, NMI, NPAD)),
            op=Alu.is_equal)

        # ---- radial early work (overlaps the gather phase) ----
        sinr = pool.tile([A, NMI], FP32)
        dcl = pool.tile([A, NMI], FP32)
        v.tensor_scalar(dcl[:, :], dpair[:, :], RCR, None, op0=Alu.min)
        i_sinr = sc.activation(sinr[:, :], dcl[:, :], Act.Sin,
                               bias=np.pi / 2, scale=-np.pi / RCR)
        add_dep_helper(i_sinr.ins, i_sqrt.ins, sync=False,
                       reason="act table order")
        fcr = pool.tile([A, NMI], FP32)
        v.tensor_scalar(fcr[:, :], sinr[:, :], 0.125, 0.125,
                        op0=Alu.mult, op1=Alu.add)
        gr = pool.tile([A, NMI], FP32)
        v.tensor_tensor(gr[:, :], fcr[:, :], maskR[:, :], op=Alu.mult)
        RT = pool.tile([A, NMI * 16], FP32)
        v.tensor_tensor(
            RT[:, :].rearrange("p (mi r) -> p mi r", r=16),
            dpair[:, :].unsqueeze(2).broadcast_to((A, NMI, 16)),
            rhoR.unsqueeze(1).broadcast_to((A, NMI, 16)),
            op=Alu.subtract)
        sc.activation(RT[:, :], RT[:, :], Act.Square, scale=4.0)

        # ====== per-center gather matmuls, two halves so the first
        # gsb copy + expansion overlap the second half's gathers ======
        gsb = pool.tile([NPAD, NMI * NARR], FP16)
        GH = NMI // 2
        for gh in range(2):
            ps_gath = psum.tile([NPAD, GH * NARR], FP32, tag="ps", bufs=2,
                                name=f"ps_gath{gh}")
            for ci in range(GH):
                cidx = gh * GH + ci
                nc.tensor.matmul(
                    ps_gath[:, ci * NARR:(ci + 1) * NARR],
                    Nsel[:, cidx * NPAD:(cidx + 1) * NPAD],
                    D8[:, cidx * NARR:(cidx + 1) * NARR],
                    start=True, stop=True)
            sc.copy(gsb[:, gh * GH * NARR:(gh + 1) * GH * NARR],
                    ps_gath[:, :])

        # ========= slot expansion via constant one-hot matmuls =========
        # X1/X2 sbuf: [128 = (cpar*64 + t), (q, arr)], center = 2q + cpar
        X1 = pool.tile([128, QP * NARR], FP32)
        X2 = pool.tile([128, QP * NARR], FP32)
        # zero (covers the unused rows 55..63 / 119..127 of each block)
        nc.gpsimd.memset(X1[:, :], 0.0)
        nc.gpsimd.memset(X2[:, :], 0.0)
        ecs = pool.tile([NPAD, 2 * T], FP16)
        v.tensor_copy(ecs[:, :T], e1c)
        v.tensor_copy(ecs[:, T:], e2c)
        QWID = NMI * NARR // 4
        for side, ec, X in ((0, ecs[:, :T], X1), (1, ecs[:, T:], X2)):
            for hh in range(4):
                ps_x = psum.tile([T, QWID], FP32, tag="psx", bufs=2,
                                 name=f"ps_x{side}{hh}")
                nc.tensor.matmul(ps_x[:, :], ec,
                                 gsb[:, hh * QWID:(hh + 1) * QWID],
                                 start=True, stop=True)
                # quarter hh covers centers [hh*40, +40) -> q in [hh*20, +20)
                for cpar in range(2):
                    srcx = bass.AP(tensor=ps_x.tensor,
                                   offset=ps_x.offset + cpar * NARR,
                                   ap=[ps_x.ap[0], [2 * NARR, QP // 4],
                                       [1, NARR]])
                    dstx = X[cpar * 64:cpar * 64 + T,
                             hh * QP * NARR // 4:(hh + 1) * QP * NARR // 4]
                    sc.copy(dstx.rearrange("p (q a) -> p q a", a=NARR), srcx)

        def xs(X, k):
            return X[:, :].rearrange("p (q a) -> p q a", a=NARR)[:, :, k]

        # W one-hot [128, (q, p)] bf16 (early: only needs species arrays)
        P2 = 128
        pv = pool.tile([P2, QP], FP32)
        tc = pool.tile([P2, QP], FP32, tag="tc")
        v.tensor_tensor(tc[:, :], xs(X1, 4), xs(X2, 4), op=Alu.max)
        v.scalar_tensor_tensor(pv[:, :], xs(X1, 4), 0.0, tc[:, :],
                               op0=Alu.mult, op1=Alu.add)
        v.tensor_tensor(tc[:, :], xs(X1, 4), xs(X2, 4), op=Alu.min)
        v.scalar_tensor_tensor(pv[:, :], tc[:, :], 3.5, pv[:, :],
                               op0=Alu.mult, op1=Alu.add)
        v.tensor_tensor(tc[:, :], tc[:, :], tc[:, :], op=Alu.mult)
        v.scalar_tensor_tensor(pv[:, :], tc[:, :], -0.5, pv[:, :],
                               op0=Alu.mult, op1=Alu.add)
        W = pool.tile([P2, QP * 10], BF16)
        v.tensor_tensor(W[:, :].rearrange("p (q b) -> p q b", b=10),
                        pv[:, :].unsqueeze(2).broadcast_to((P2, QP, 10)),
                        io10.unsqueeze(1).broadcast_to((128, QP, 10)),
                        op=Alu.is_equal)

        # ================= slot math: [128, 80] =================
        cdot = pool.tile([P2, QP], FP32)
        ta = pool.tile([P2, QP], FP32, tag="ta")
        tb = pool.tile([P2, QP], FP32, tag="tb")
        v.tensor_tensor(cdot[:, :], xs(X1, 0), xs(X2, 0), op=Alu.mult)
        v.tensor_tensor(ta[:, :], xs(X1, 1), xs(X2, 1), op=Alu.mult)
        v.tensor_tensor(cdot[:, :], cdot[:, :], ta[:, :], op=Alu.add)
        v.tensor_tensor(ta[:, :], xs(X1, 2), xs(X2, 2), op=Alu.mult)
        v.tensor_tensor(cdot[:, :], cdot[:, :], ta[:, :], op=Alu.add)
        # cutoffs (trig set)
        sin1 = pool.tile([P2, QP], FP32)
        sin2 = pool.tile([P2, QP], FP32)
        i_sin1 = sc.activation(sin1[:, :], xs(X1, 3), Act.Sin,
                               bias=np.pi / 2, scale=-np.pi / RCA)
        i_sin2 = sc.activation(sin2[:, :], xs(X2, 3), Act.Sin,
                               bias=np.pi / 2, scale=-np.pi / RCA)
        for _t in (i_sin1, i_sin2):
            add_dep_helper(_t.ins, i_sqrt.ins, sync=False,
                           reason="act table order")
        _sins = (i_sin1, i_sin2, i_sinr)

        def lnexp(*args, **kw):
            ins = sc.activation(*args, **kw)
            for _s in _sins:
                add_dep_helper(ins.ins, _s.ins, sync=False,
                               reason="act table order")
            return ins
        # w = fc1*fc2*valid1*valid2
        w = pool.tile([P2, QP], FP32)
        v.tensor_scalar(ta[:, :], sin1[:, :], 0.5, 0.5, op0=Alu.mult,
                        op1=Alu.add)
        v.tensor_scalar(tb[:, :], sin2[:, :], 0.5, 0.5, op0=Alu.mult,
                        op1=Alu.add)
        v.tensor_tensor(w[:, :], ta[:, :], tb[:, :], op=Alu.mult)
        v.tensor_tensor(ta[:, :], xs(X1, 5), xs(X2, 5), op=Alu.mult)
        v.tensor_tensor(w[:, :], w[:, :], ta[:, :], op=Alu.mult)
        # s = sqrt(1 - (0.95 c)^2) via ln/exp
        csq = pool.tile([P2, QP], FP32)
        sc.activation(csq[:, :], cdot[:, :], Act.Square, scale=0.95)
        svar = pool.tile([P2, QP], FP32)
        lnexp(svar[:, :], csq[:, :], Act.Ln, bias=1.0, scale=-1.0)
        lnexp(svar[:, :], svar[:, :], Act.Exp, scale=0.5)
        # f2 = 2*exp(-2(u-2rho)^2) * w   [128, (a, q)]
        u = pool.tile([P2, QP], FP32)
        v.tensor_tensor(u[:, :], xs(X1, 3), xs(X2, 3), op=Alu.add)
        f2 = pool.tile([P2, 4 * QP], FP32)
        v.tensor_tensor(
            f2[:, :].rearrange("p (a q) -> p a q", a=4),
            u[:, :].unsqueeze(1).broadcast_to((P2, 4, QP)),
            rho2.unsqueeze(2).broadcast_to((128, 4, QP)),
            op=Alu.subtract)
        sc.activation(f2[:, :], f2[:, :], Act.Square, scale=float(np.sqrt(2.0)))
        lnexp(f2[:, :], f2[:, :], Act.Exp, bias=LN2, scale=-1.0)
        v.tensor_tensor(
            f2[:, :].rearrange("p (a q) -> p a q", a=4),
            f2[:, :].rearrange("p (a q) -> p a q", a=4),
            w[:, :].unsqueeze(1).broadcast_to((P2, 4, QP)),
            op=Alu.mult)
        # f1 = ((1 + 0.95 c cos(phi_z) + s sin(phi_z))/2)^32   [128, (z, q)]
        azv = pool.tile([P2, 8 * QP], FP32)
        HF = QP // 2
        for fh in range(2):
            for z in range(8):
                ccz = float(0.475 * np.cos(SHF_Z[z]))
                ssz = float(0.5 * np.sin(SHF_Z[z]))
                sl = azv[:, z * QP + fh * HF:z * QP + (fh + 1) * HF]
                cs = cdot[:, fh * HF:(fh + 1) * HF]
                ss = svar[:, fh * HF:(fh + 1) * HF]
                v.tensor_scalar(sl, cs, ccz, 0.5, op0=Alu.mult, op1=Alu.add)
                v.scalar_tensor_tensor(sl, ss, ssz, sl,
                                       op0=Alu.mult, op1=Alu.add)
            hap = bass.AP(tensor=azv.tensor,
                          offset=azv[:, :].offset + fh * HF,
                          ap=[azv[:, :].ap[0], [QP, 8], [1, HF]])
            lnexp(hap, hap, Act.Ln, bias=1e-30, scale=1.0)
            lnexp(hap, hap, Act.Exp, scale=float(ZETA))
        # F [128, (q, a, z)] bf16
        F = pool.tile([P2, QP * 32], BF16)
        for fh in range(2):
            HF = QP // 2
            ap_f2 = bass.AP(tensor=f2.tensor,
                            offset=f2[:, :].offset + fh * HF,
                            ap=[f2[:, :].ap[0], [1, HF], [QP, 4], [0, 8]])
            ap_f1 = bass.AP(tensor=azv.tensor,
                            offset=azv[:, :].offset + fh * HF,
                            ap=[azv[:, :].ap[0], [1, HF], [0, 4], [QP, 8]])
            fdst = F[:, fh * HF * 32:(fh + 1) * HF * 32]
            v.tensor_tensor(fdst.rearrange("p (q a z) -> p q a z", a=4, z=8),
                            ap_f2, ap_f1, op=Alu.mult)
        # ================= angular bin-reduce =================
        # out[p, az] per center; one PSUM round per cpar (base 0, then 64).
        # out_sb spreads mi%4 over partition groups 0/32/64/96 so the four
        # output DMAs use different SBUF port groups in parallel.
        out_sb = pool.tile([128, (NMI // 4) * 32], FP32)
        HQ = QP // 2
        for rnd in range(4):
            cpar, hh = rnd % 2, rnd // 2
            ps_bin = psum.tile([10, HQ * 32], FP32, tag="ps", bufs=2,
                               name=f"ps_bin{rnd}")
            for qi in range(HQ):
                q = hh * HQ + qi
                nc.tensor.matmul(
                    ps_bin[:, qi * 32:(qi + 1) * 32],
                    W[cpar * 64:cpar * 64 + T, q * 10:(q + 1) * 10],
                    F[cpar * 64:cpar * 64 + T, q * 32:(q + 1) * 32],
                    start=True, stop=True)
            # center = 2q + cpar; q parity splits mi%4 = cpar / cpar+2
            for qpar in range(2):
                k = 2 * qpar + cpar            # = mi % 4
                dstk = bass.AP(
                    tensor=out_sb.tensor,
                    offset=(out_sb[:, :].offset + 32 * k * out_sb.ap[0][0]
                            + hh * (HQ // 2) * 32),
                    ap=[[out_sb.ap[0][0], 10], [32, HQ // 2], [1, 32]])
                srck = bass.AP(tensor=ps_bin.tensor,
                               offset=ps_bin[:, :].offset + qpar * 32,
                               ap=[ps_bin[:, :].ap[0], [64, HQ // 2], [1, 32]])
                eng = v.tensor_copy if qpar == 0 else sc.copy
                eng(dstk, srck)
        # four DMAs (one per mi%4 group, different ports/queues)
        for k in range(4):
            dst_ang = bass.AP(tensor=out_e[:, :, :].tensor,
                              offset=64 + k * 384,
                              ap=[[32, 10], [4 * 384, NMI // 4], [1, 32]])
            base = out_sb[32 * k:32 * k + 10, :]
            src_ang = bass.AP(tensor=base.tensor, offset=base.offset,
                              ap=[base.ap[0], [32, NMI // 4], [1, 32]])
            eng = (nc.sync, nc.scalar, nc.gpsimd, nc.sync)[k]
            eng.dma_start(out=dst_ang, in_=src_ang)

        # ================= radial =================
        lngr = pool.tile([A, NMI], FP32)
        lnexp(lngr[:, :], gr[:, :], Act.Ln, bias=1e-35, scale=1.0)
        v.scalar_tensor_tensor(
            RT[:, :].rearrange("p (mi r) -> p mi r", r=16),
            RT[:, :].rearrange("p (mi r) -> p mi r", r=16), -1.0,
            lngr[:, :].unsqueeze(2).broadcast_to((A, NMI, 16)),
            op0=Alu.mult, op1=Alu.add)
        lnexp(RT[:, :], RT[:, :], Act.Exp)
        rad_sb = pool.tile([4, NMI * 16], FP32)
        for m in range(M4):
            for h in range(2):
                ps_rad = psum.tile([4, 512], FP32, tag="psx", bufs=2,
                                   name=f"ps_rad{m}{h}")
                nc.tensor.matmul(
                    ps_rad[:, :320],
                    ohm[:, m * 4:(m + 1) * 4],
                    RT[:, (m * 640 + h * 320):(m * 640 + (h + 1) * 320)],
                    start=True, stop=True)
                eng = sc.copy if (m * 2 + h) % 2 == 0 else v.tensor_copy
                eng(rad_sb[:, m * 640 + h * 320:m * 640 + (h + 1) * 320],
                    ps_rad[:, :320])
        dst_rad = bass.AP(tensor=out_e[:, :, :].tensor, offset=0,
                          ap=[[16, 4], [384, NMI], [1, 16]])
        src_rad = bass.AP(tensor=rad_sb.tensor, offset=rad_sb[:, :].offset,
                          ap=[rad_sb[:, :].ap[0], [16, NMI], [1, 16]])
        nc.sync.dma_start(out=dst_rad, in_=src_rad)

    nc.finalize()
    _NC_CACHE["nc"] = nc
    return nc


def make_in_maps(coords, elem):
    """Host-side sharding + layout prep (no physics)."""
    coords = np.asarray(coords, dtype=np.float32).reshape(M, A, 3)
    elem = np.asarray(elem).reshape(M, A)
    in_maps = []
    lt = (np.arange(A)[:, None] < np.arange(A)[None, :]).astype(np.float32)
    neq = np.ones((A, NMI), np.float32)
    for i in range(A):
        for m in range(M4):
            neq[i, m * A + i] = 0.0
    io16 = np.broadcast_to(np.arange(NPAD, dtype=np.float32), (A, NPAD))
    io10 = np.broadcast_to(np.arange(10, dtype=np.float32), (128, 10))
    rho2 = np.broadcast_to(2.0 * SHF_A, (128, 4))
    rhoR = np.broadcast_to(SHF_R, (A, 16))
    e1 = (N1[None, :] == np.arange(NPAD)[:, None]).astype(np.float32)
    e2 = (N2[None, :] == np.arange(NPAD)[:, None]).astype(np.float32)
    for c in range(N_CORES):
        cm = coords[c * M4:(c + 1) * M4]          # [M4, A, 3]
        em = elem[c * M4:(c + 1) * M4]            # [M4, A]
        xj = cm.transpose(1, 0, 2).reshape(A, M4 * 3)
        xi = np.broadcast_to(cm.reshape(1, NMI * 3), (A, NMI * 3))
        ohm = np.zeros((A, M4 * 4), np.float32)
        for m in range(M4):
            for s in range(4):
                ohm[:, m * 4 + s] = (em[m] == s)
        sj = em.transpose(1, 0).astype(np.float32)
        vals = {"xj": xj, "xi": xi, "ohm": ohm, "sj": sj, "lt": lt,
                "neq": neq, "io16": io16, "rhoR": rhoR, "e1": e1, "e2": e2,
                "io10": io10, "rho2": rho2}
        blob = np.zeros((128, CST_COLS), np.float32)
        o = 0
        for nm, wd in CST_LAYOUT_A + CST_LAYOUT_F:
            arr = vals[nm]
            blob[:arr.shape[0], o:o + wd] = arr
            o += wd
        in_maps.append({"cst": blob})
    return in_maps


def kernel(elem_idxs, coords):
    elem_idxs = np.asarray(elem_idxs)
    coords = np.asarray(coords)
    nc = build_nc()
    in_maps = make_in_maps(coords, elem_idxs)
    res = run_bass_kernel_spmd(nc, in_maps, core_ids=list(range(N_CORES)))
    outs = [res.results[c]["out"] for c in range(N_CORES)]
    return np.concatenate(outs, axis=0).astype(np.float32)



# revision 10
# speedup vs baseline: 1.2084x; 1.2084x over previous
"""AEV (ANI-1x) computer on 8 Trainium2 NeuronCores.

Data-parallel over molecules: each core computes 4 of the 32 molecules.
v2: activation-table grouping (sqrt/sin/ln/exp phases), 3-op azv build,
fp16 radial matmuls scheduled early, 2-center-stacked bin-reduce (80
matmuls) with PSUM-direct output DMAs, 2x-mode one-hot builds.
"""
import sys
import numpy as np

sys.path.insert(0, "/opt/trn_rl_repo")

from concourse import bacc, bass, mybir, tile  # noqa: E402
from concourse.tile import add_dep_helper  # noqa: E402
from concourse.bass_utils import run_bass_kernel_spmd  # noqa: E402

# ---- problem constants ----
M, A = 32, 40
N_CORES = 8
M4 = M // N_CORES           # molecules per core
NMI = M4 * A                # (m, i) centers per core = 160
RCR, RCA = 5.2, 3.5
ETA_R, ETA_A, ZETA = 16.0, 8.0, 32.0
SHF_R = np.linspace(0.9, RCR, 17, dtype=np.float32)[:-1]    # 16
SHF_A = np.array([0.9, 1.55, 2.2, 2.85], dtype=np.float32)  # 4
SHF_Z = (np.pi / 16.0) * (2.0 * np.arange(8, dtype=np.float32) + 1.0)
NMAX = 11                   # max angular neighbors supported (data max = 10)
NPAD = 16                   # padded neighbor slots for the gather matmul
T = NMAX * (NMAX - 1) // 2  # 55 pair-slots per center
N1 = np.array([a for a in range(NMAX) for b in range(a + 1, NMAX)])
N2 = np.array([b for a in range(NMAX) for b in range(a + 1, NMAX)])
NARR = 8                    # gathered arrays: ux,uy,uz,d,spec,valid,0,0
QP = NMI // 2               # center-pairs in packed slot layout = 80
LN2 = float(np.log(2.0))
FP32 = mybir.dt.float32
FP16 = mybir.dt.float16
Alu = mybir.AluOpType
Act = mybir.ActivationFunctionType

# fp32 constant blob layout (rows x cols); A-rows section then 128-rows
CST_LAYOUT_A = [("xj", M4 * 3), ("xi", NMI * 3), ("sj", M4), ("lt", A),
                ("neq", NMI), ("rhoR", 16)]
CST_LAYOUT_F = [("rho2", 4), ("ccz", 8), ("ssz", 8)]
CST_A_COLS = sum(w for _, w in CST_LAYOUT_A)
CST_COLS = CST_A_COLS + sum(w for _, w in CST_LAYOUT_F)

# fp16 constant blob layout
CSTH_LAYOUT = [("io_nmi", NPAD * NMI), ("io20", 20 * QP), ("e1", T),
               ("e2", T), ("ohmf", 16)]
CSTH_COLS = sum(w for _, w in CSTH_LAYOUT)

_NC_CACHE = {}


def build_nc():
    if "nc" in _NC_CACHE:
        return _NC_CACHE["nc"]
    from contextlib import ExitStack
    nc = bacc.Bacc()
    cst_e = nc.declare_dram_parameter("cst", [128, CST_COLS], FP32,
                                      isOutput=False)
    csth_e = nc.declare_dram_parameter("csth", [128, CSTH_COLS], FP16,
                                       isOutput=False)
    out_e = nc.declare_dram_parameter("out", [M4, A, 384], FP32, isOutput=True)

    with tile.TileContext(nc) as tc, ExitStack() as es:
        pool = es.enter_context(tc.tile_pool(name="sb", bufs=1))
        psum = es.enter_context(tc.tile_pool(name="ps", bufs=1, space="PSUM"))
        v = nc.vector
        sc = nc.scalar
        gp = nc.gpsimd

        # activation bias constants
        for cval in (1e-12, float(np.pi / 2), 1.0, 0.5 + 1e-7, LN2, 1e-35):
            cpk = (FP32, cval)
            if cpk not in nc.const_aps.aps:
                ct = pool.tile([128, 1], FP32, name=f"cst{len(nc.const_aps.aps)}")
                v.memset(ct[:, :], cval)
                nc.const_aps.aps[cpk] = ct

        # ---- load input/constant blobs across queues ----
        cst = pool.tile([128, CST_COLS], FP32)
        HC = CST_A_COLS // 2
        nc.sync.dma_start(out=cst[:A, :HC], in_=cst_e[:A, :HC])
        nc.scalar.dma_start(out=cst[:A, HC:CST_A_COLS],
                            in_=cst_e[:A, HC:CST_A_COLS])
        nc.scalar.dma_start(out=cst[:, CST_A_COLS:], in_=cst_e[:, CST_A_COLS:])
        csth = pool.tile([128, CSTH_COLS], FP16)
        HH = NPAD * NMI  # io_nmi section (rows :A)
        nc.sync.dma_start(out=csth[:A, :HH], in_=csth_e[:A, :HH])
        nc.gpsimd.dma_start(out=csth[:, HH:], in_=csth_e[:, HH:])

        off = {}
        o = 0
        for nm, wd in CST_LAYOUT_A + CST_LAYOUT_F:
            off[nm] = o
            o += wd
        offh = {}
        o = 0
        for nm, wd in CSTH_LAYOUT:
            offh[nm] = o
            o += wd

        def cv(nm, rows, wd):
            return cst[0:rows, off[nm]:off[nm] + wd]

        def cvh(nm, rows, wd):
            return csth[0:rows, offh[nm]:offh[nm] + wd]

        xj = cv("xj", A, M4 * 3)
        xi = cv("xi", A, NMI * 3)
        sj = cv("sj", A, M4)
        lt = cv("lt", A, A)
        neq = cv("neq", A, NMI)
        rhoR = cv("rhoR", A, 16)
        rho2 = cv("rho2", 128, 4)
        cczC = cv("ccz", 128, 8)
        sszC = cv("ssz", 128, 8)
        io_nmi = cvh("io_nmi", A, NPAD * NMI)     # [A, (n, mi)] = n
        io20b = cvh("io20", 128, 20 * QP)         # [128, (b', q)] = b'-10*(p>=64)
        e1c = cvh("e1", NPAD, T)
        e2c = cvh("e2", NPAD, T)
        ohmf = cvh("ohmf", A, 16)                 # [A, (m, s)] one-hot fp16

        # activation-table group bookkeeping: all T-group acts depend on all
        # S-group acts, etc.  Groups: S=sqrt, T=sin, L=ln, X=exp.
        act_groups = {"S": [], "T": [], "L": [], "X": []}
        order = ["S", "T", "L", "X"]

        def act(group, *args, **kw):
            ins = sc.activation(*args, **kw)
            gi = order.index(group)
            for gprev in order[:gi]:
                for prev in act_groups[gprev]:
                    add_dep_helper(ins.ins, prev.ins, sync=False,
                                   reason="act table order")
            act_groups[group].append(ins)
            return ins

        # ================= pair stage: [40 j, 160 (m,i)] =================
        D8 = pool.tile([A, NMI * NARR], FP16)   # gather-matmul rhs
        d8pad = bass.AP(tensor=D8.tensor, offset=D8[:, :].offset + 6,
                        ap=[D8[:, :].ap[0], [NARR, NMI], [1, 2]])
        gp.memset(d8pad, 0.0)

        def d8slot(k):
            return D8[:, :].rearrange("p (mi a) -> p mi a", a=NARR)[:, :, k]

        dx = [pool.tile([A, NMI], FP32, name=f"dx{c}", tag=f"dx{c}")
              for c in range(3)]
        for c in range(3):
            in_j = bass.AP(tensor=xj.tensor, offset=xj.offset + c,
                           ap=[xj.ap[0], [3, M4], [0, A]])
            in_i = bass.AP(tensor=xi.tensor, offset=xi.offset + c,
                           ap=[xi.ap[0], [3 * A, M4], [3, A]])
            v.tensor_tensor(dx[c][:, :].rearrange("p (m i) -> p m i", m=M4),
                            in_j, in_i, op=Alu.subtract)
        dsq = pool.tile([A, NMI], FP32)
        t0 = pool.tile([A, NMI], FP32, tag="t0")
        t1 = pool.tile([A, NMI], FP32, tag="t1")
        v.tensor_tensor(t0[:, :], dx[0][:, :], dx[0][:, :], op=Alu.mult)
        v.tensor_tensor(t1[:, :], dx[1][:, :], dx[1][:, :], op=Alu.mult)
        v.tensor_tensor(t0[:, :], t0[:, :], t1[:, :], op=Alu.add)
        v.tensor_tensor(t1[:, :], dx[2][:, :], dx[2][:, :], op=Alu.mult)
        v.tensor_tensor(dsq[:, :], t0[:, :], t1[:, :], op=Alu.add)
        # masks
        maskA = pool.tile([A, NMI], FP32)
        maskR = pool.tile([A, NMI], FP32)
        v.tensor_scalar(t0[:, :], dsq[:, :], RCA * RCA, None, op0=Alu.is_lt)
        v.tensor_tensor(maskA[:, :], t0[:, :], neq, op=Alu.mult)
        v.tensor_scalar(t1[:, :], dsq[:, :], RCR * RCR, None, op0=Alu.is_lt)
        v.tensor_tensor(maskR[:, :], t1[:, :], neq, op=Alu.mult)
        v.tensor_copy(d8slot(5), maskA[:, :])
        # d (sqrt, S group), 1/d, unit vectors
        dpair = pool.tile([A, NMI], FP32)
        act("S", dpair[:, :], dsq[:, :], Act.Sqrt, bias=1e-12, scale=1.0)
        inv = pool.tile([A, NMI], FP32)
        v.reciprocal(inv[:, :], dpair[:, :])
        for c in range(3):
            v.tensor_tensor(d8slot(c), dx[c][:, :], inv[:, :], op=Alu.mult)
        v.tensor_copy(d8slot(3), dpair[:, :])
        # species of j replicated along i
        in_s = bass.AP(tensor=sj.tensor, offset=sj.offset,
                       ap=[sj.ap[0], [1, M4], [0, A]])
        v.tensor_copy(d8slot(4).rearrange("p (m i) -> p m i", m=M4), in_s)

        # ================= neighbor ranks via PE =================
        ps_rank = psum.tile([A, NMI], FP32, tag="ps", bufs=2)
        nc.tensor.matmul(ps_rank[:, :], lt, maskA[:, :], start=True, stop=True)
        rankp = pool.tile([A, NMI], FP32)
        rankp16 = pool.tile([A, NMI], FP16)
        # valid j -> rank (0..10); invalid -> rank - 1000
        v.scalar_tensor_tensor(rankp[:, :], maskA[:, :], 1000.0, ps_rank[:, :],
                               op0=Alu.mult, op1=Alu.add)
        v.tensor_scalar(rankp16[:, :], rankp[:, :], 1000.0, None,
                        op0=Alu.subtract)
        # one-hot Nsel [40, (n, mi)] fp16 n-major (2x DVE mode: all operands
        # fp16 with packed inner mi)
        Nsel = pool.tile([A, NPAD * NMI], FP16)
        v.tensor_tensor(
            Nsel[:, :].rearrange("p (n mi) -> p n mi", n=NPAD),
            rankp16[:, :].unsqueeze(1).broadcast_to((A, NPAD, NMI)),
            io_nmi.rearrange("p (n mi) -> p n mi", n=NPAD),
            op=Alu.is_equal)

        # ---- radial early work (overlaps the gather phase) ----
        dcl = pool.tile([A, NMI], FP32)
        v.tensor_scalar(dcl[:, :], dpair[:, :], RCR, None, op0=Alu.min)
        RT = pool.tile([A, NMI * 16], FP32)
        v.tensor_tensor(
            RT[:, :].rearrange("p (mi r) -> p mi r", r=16),
            dpair[:, :].unsqueeze(2).broadcast_to((A, NMI, 16)),
            rhoR.unsqueeze(1).broadcast_to((A, NMI, 16)),
            op=Alu.subtract)
        sc.activation(RT[:, :], RT[:, :], Act.Square, scale=4.0)

        # ====== per-center gather matmuls, two halves so the first
        # gsb copy + expansion overlap the second half's gathers ======
        gsb = pool.tile([NPAD, NMI * NARR], FP16)
        GH = NMI // 2
        for gh in range(2):
            ps_gath = psum.tile([NPAD, GH * NARR], FP32, tag="ps", bufs=2,
                                name=f"ps_gath{gh}")
            for ci in range(GH):
                cidx = gh * GH + ci
                lhs = bass.AP(tensor=Nsel.tensor,
                              offset=Nsel[:, :].offset + cidx,
                              ap=[Nsel[:, :].ap[0], [NMI, NPAD]])
                nc.tensor.matmul(
                    ps_gath[:, ci * NARR:(ci + 1) * NARR],
                    lhs,
                    D8[:, cidx * NARR:(cidx + 1) * NARR],
                    start=True, stop=True)
            eng = sc.copy if gh == 0 else v.tensor_copy
            eng(gsb[:, gh * GH * NARR:(gh + 1) * GH * NARR], ps_gath[:, :])

        # ========= slot expansion via constant one-hot matmuls =========
        # X1/X2 sbuf: [128 = (cpar*64 + t), (q, arr)], center = 2q + cpar
        X1 = pool.tile([128, QP * NARR], FP32)
        X2 = pool.tile([128, QP * NARR], FP32)
        # zero (covers the unused rows 55..63 / 119..127 of each block)
        gp.memset(X1[:, :], 0.0)
        gp.memset(X2[:, :], 0.0)
        QWID = NMI * NARR // 4
        xcopy_engs = [sc.copy, v.tensor_copy]
        xci = 0
        for side, ec, X in ((0, e1c, X1), (1, e2c, X2)):
            for hh in range(4):
                ps_x = psum.tile([T, QWID], FP32, tag="psx", bufs=2,
                                 name=f"ps_x{side}{hh}")
                nc.tensor.matmul(ps_x[:, :], ec,
                                 gsb[:, hh * QWID:(hh + 1) * QWID],
                                 start=True, stop=True)
                # quarter hh covers centers [hh*40, +40) -> q in [hh*20, +20)
                for cpar in range(2):
                    srcx = bass.AP(tensor=ps_x.tensor,
                                   offset=ps_x.offset + cpar * NARR,
                                   ap=[ps_x.ap[0], [2 * NARR, QP // 4],
                                       [1, NARR]])
                    dstx = X[cpar * 64:cpar * 64 + T,
                             hh * QP * NARR // 4:(hh + 1) * QP * NARR // 4]
                    xcopy_engs[xci % len(xcopy_engs)](
                        dstx.rearrange("p (q a) -> p q a", a=NARR), srcx)
                    xci += 1

        def xs(X, k):
            return X[:, :].rearrange("p (q a) -> p q a", a=NARR)[:, :, k]

        # ================= slot math: [128, 80] =================
        P2 = 128
        cdot = pool.tile([P2, QP], FP32)
        ta = pool.tile([P2, QP], FP32, tag="ta")
        tb = pool.tile([P2, QP], FP32, tag="tb")
        v.tensor_tensor(cdot[:, :], xs(X1, 0), xs(X2, 0), op=Alu.mult)
        v.tensor_tensor(ta[:, :], xs(X1, 1), xs(X2, 1), op=Alu.mult)
        v.tensor_tensor(cdot[:, :], cdot[:, :], ta[:, :], op=Alu.add)
        v.tensor_tensor(ta[:, :], xs(X1, 2), xs(X2, 2), op=Alu.mult)
        v.tensor_tensor(cdot[:, :], cdot[:, :], ta[:, :], op=Alu.add)
        # s = sqrt(1 - (0.95 c)^2) -- S group (before the sins)
        csq = pool.tile([P2, QP], FP32)
        sc.activation(csq[:, :], cdot[:, :], Act.Square, scale=0.95)
        svar = pool.tile([P2, QP], FP32)
        act("S", svar[:, :], csq[:, :], Act.Sqrt, bias=1.0, scale=-1.0)
        # cutoffs (trig group)
        sinr = pool.tile([A, NMI], FP32)
        act("T", sinr[:, :], dcl[:, :], Act.Sin, bias=np.pi / 2,
            scale=-np.pi / RCR)
        sin1 = pool.tile([P2, QP], FP32)
        sin2 = pool.tile([P2, QP], FP32)
        act("T", sin1[:, :], xs(X1, 3), Act.Sin, bias=np.pi / 2,
            scale=-np.pi / RCA)
        act("T", sin2[:, :], xs(X2, 3), Act.Sin, bias=np.pi / 2,
            scale=-np.pi / RCA)
        # w = fc1*fc2*valid1*valid2
        w = pool.tile([P2, QP], FP32)
        v.tensor_scalar(ta[:, :], sin1[:, :], 0.5, 0.5, op0=Alu.mult,
                        op1=Alu.add)
        v.tensor_scalar(tb[:, :], sin2[:, :], 0.5, 0.5, op0=Alu.mult,
                        op1=Alu.add)
        v.tensor_tensor(w[:, :], ta[:, :], tb[:, :], op=Alu.mult)
        v.tensor_tensor(ta[:, :], xs(X1, 5), xs(X2, 5), op=Alu.mult)
        v.tensor_tensor(w[:, :], w[:, :], ta[:, :], op=Alu.mult)
        # radial fc chain (vector) for lngr
        fcr = pool.tile([A, NMI], FP32)
        v.tensor_scalar(fcr[:, :], sinr[:, :], 0.125, 0.125,
                        op0=Alu.mult, op1=Alu.add)
        gr = pool.tile([A, NMI], FP32)
        v.tensor_tensor(gr[:, :], fcr[:, :], maskR[:, :], op=Alu.mult)

        # f2 = 2*exp(-2(u-2rho)^2) * w   [128, (q, a)] q-major
        u = pool.tile([P2, QP], FP32)
        v.tensor_tensor(u[:, :], xs(X1, 3), xs(X2, 3), op=Alu.add)
        f2 = pool.tile([P2, QP * 4], FP32)
        v.tensor_tensor(
            f2[:, :].rearrange("p (q a) -> p q a", a=4),
            u[:, :].unsqueeze(2).broadcast_to((P2, QP, 4)),
            rho2.unsqueeze(1).broadcast_to((128, QP, 4)),
            op=Alu.subtract)
        sc.activation(f2[:, :], f2[:, :], Act.Square, scale=float(np.sqrt(2.0)))

        # azv = 0.5 + 0.475 cos(phi_z) c + 0.5 sin(phi_z) s  [128, (q, z)]
        azv = pool.tile([P2, QP * 8], FP32)
        tz = pool.tile([P2, QP * 8], FP32)
        v.tensor_tensor(
            azv[:, :].rearrange("p (q z) -> p q z", z=8),
            cdot[:, :].unsqueeze(2).broadcast_to((P2, QP, 8)),
            cczC.unsqueeze(1).broadcast_to((128, QP, 8)),
            op=Alu.mult)
        v.tensor_tensor(
            tz[:, :].rearrange("p (q z) -> p q z", z=8),
            svar[:, :].unsqueeze(2).broadcast_to((P2, QP, 8)),
            sszC.unsqueeze(1).broadcast_to((128, QP, 8)),
            op=Alu.mult)
        v.tensor_tensor(azv[:, :], azv[:, :], tz[:, :], op=Alu.add)
        # ln phase (L group): azv ln, lngr
        act("L", azv[:, :], azv[:, :], Act.Ln, bias=0.5 + 1e-7, scale=1.0)
        lngr = pool.tile([A, NMI], FP32)
        act("L", lngr[:, :], gr[:, :], Act.Ln, bias=1e-35, scale=1.0)
        # exp phase (X group): azvb = f1^(1) in fp16, f2 exp, RT exp
        azvb = pool.tile([P2, QP * 8], FP16)
        act("X", azvb[:, :], azv[:, :], Act.Exp, scale=float(ZETA))
        act("X", f2[:, :], f2[:, :], Act.Exp, bias=LN2, scale=-1.0)
        # merge radial: RT = exp(-RT + lngr)
        v.scalar_tensor_tensor(
            RT[:, :].rearrange("p (mi r) -> p mi r", r=16),
            RT[:, :].rearrange("p (mi r) -> p mi r", r=16), -1.0,
            lngr[:, :].unsqueeze(2).broadcast_to((A, NMI, 16)),
            op0=Alu.mult, op1=Alu.add)
        RTf = pool.tile([A, NMI * 16], FP16)
        act("X", RTf[:, :], RT[:, :], Act.Exp)

        # ---- radial matmuls (fp16, emitted before bin-reduce) ----
        rad_sb = pool.tile([4, NMI * 16], FP32)
        rad_engs = [sc.copy, v.tensor_copy, sc.copy, v.tensor_copy,
                    sc.copy, v.tensor_copy, sc.copy, v.tensor_copy]
        for m in range(M4):
            for h in range(2):
                ps_rad = psum.tile([4, 320], FP32, tag="psx", bufs=2,
                                   name=f"ps_rad{m}{h}")
                nc.tensor.matmul(
                    ps_rad[:, :320],
                    ohmf[:, m * 4:(m + 1) * 4],
                    RTf[:, (m * 640 + h * 320):(m * 640 + (h + 1) * 320)],
                    start=True, stop=True)
                rad_engs[m * 2 + h](
                    rad_sb[:, m * 640 + h * 320:m * 640 + (h + 1) * 320],
                    ps_rad[:, :320])
        dst_rad = bass.AP(tensor=out_e[:, :, :].tensor, offset=0,
                          ap=[[16, 4], [384, NMI], [1, 16]])
        src_rad = bass.AP(tensor=rad_sb.tensor, offset=rad_sb[:, :].offset,
                          ap=[rad_sb[:, :].ap[0], [16, NMI], [1, 16]])
        nc.sync.dma_start(out=dst_rad, in_=src_rad)

        # f2 *= w; write fp16 for the F build
        f2b = pool.tile([P2, QP * 4], FP16)
        v.tensor_tensor(
            f2b[:, :].rearrange("p (q a) -> p q a", a=4),
            f2[:, :].rearrange("p (q a) -> p q a", a=4),
            w[:, :].unsqueeze(2).broadcast_to((P2, QP, 4)),
            op=Alu.mult)

        # F [128, (q, a, z)] fp16 = f2b (q,a) x azvb (q,z)
        F = pool.tile([P2, QP * 32], FP16)
        HF = QP // 2
        for fh in range(2):
            ap_f2 = bass.AP(tensor=f2b.tensor,
                            offset=f2b[:, :].offset + fh * HF * 4,
                            ap=[f2b[:, :].ap[0], [4, HF], [1, 4], [0, 8]])
            ap_f1 = bass.AP(tensor=azvb.tensor,
                            offset=azvb[:, :].offset + fh * HF * 8,
                            ap=[azvb[:, :].ap[0], [8, HF], [0, 4], [1, 8]])
            fdst = F[:, fh * HF * 32:(fh + 1) * HF * 32]
            v.tensor_tensor(fdst.rearrange("p (q a z) -> p q a z", a=4, z=8),
                            ap_f2, ap_f1, op=Alu.mult)

        # W2 one-hot [128, (b', q)] fp16, b' = bin + 10*cpar via io20b
        pv = pool.tile([P2, QP], FP32)
        tc2 = pool.tile([P2, QP], FP32, tag="tc")
        pv16 = pool.tile([P2, QP], FP16)
        v.tensor_tensor(tc2[:, :], xs(X1, 4), xs(X2, 4), op=Alu.max)
        v.scalar_tensor_tensor(pv[:, :], xs(X1, 4), 0.0, tc2[:, :],
                               op0=Alu.mult, op1=Alu.add)
        v.tensor_tensor(tc2[:, :], xs(X1, 4), xs(X2, 4), op=Alu.min)
        v.scalar_tensor_tensor(pv[:, :], tc2[:, :], 3.5, pv[:, :],
                               op0=Alu.mult, op1=Alu.add)
        v.tensor_tensor(tc2[:, :], tc2[:, :], tc2[:, :], op=Alu.mult)
        v.scalar_tensor_tensor(pv16[:, :], tc2[:, :], -0.5, pv[:, :],
                               op0=Alu.mult, op1=Alu.add)
        W2 = pool.tile([P2, 20 * QP], FP16)
        v.tensor_tensor(
            W2[:, :].rearrange("p (b q) -> p b q", b=20),
            pv16[:, :].unsqueeze(1).broadcast_to((P2, 20, QP)),
            io20b.rearrange("p (b q) -> p b q", b=20),
            op=Alu.is_equal)

        # ================= angular bin-reduce =================
        # 2-center-stacked: one matmul per q -> psum [20, 32]; DMA rounds
        # straight from PSUM to HBM (2 DMAs per round, one per parity).
        RQ = 16                      # q per round
        out_sb = pool.tile([20, QP * 32], FP32)
        dma_qs = [nc.sync, nc.scalar, nc.gpsimd]
        cp_engs = [v.tensor_copy, sc.copy]
        for rnd in range(QP // RQ):
            ps_bin = psum.tile([20, RQ * 32], FP32, tag="ps", bufs=2,
                               name=f"ps_bin{rnd}")
            for qi in range(RQ):
                q = rnd * RQ + qi
                lhs = bass.AP(tensor=W2.tensor,
                              offset=W2[:, :].offset + q,
                              ap=[W2[:, :].ap[0], [QP, 20]])
                nc.tensor.matmul(
                    ps_bin[:, qi * 32:(qi + 1) * 32],
                    lhs,
                    F[:, q * 32:(q + 1) * 32],
                    start=True, stop=True)
            sl = out_sb[:, rnd * RQ * 32:(rnd + 1) * RQ * 32]
            cp_engs[rnd % len(cp_engs)](sl, ps_bin[:, :])
            for cpar in range(2):
                dstb = bass.AP(tensor=out_e[:, :, :].tensor,
                               offset=(2 * RQ * rnd + cpar) * 384 + 64,
                               ap=[[32, 10], [2 * 384, RQ], [1, 32]])
                sb_sub = out_sb[cpar * 10:cpar * 10 + 10,
                                rnd * RQ * 32:(rnd + 1) * RQ * 32]
                srcb = bass.AP(tensor=sb_sub.tensor, offset=sb_sub.offset,
                               ap=[sb_sub.ap[0], [32, RQ], [1, 32]])
                dma_qs[(rnd * 2 + cpar) % len(dma_qs)].dma_start(
                    out=dstb, in_=srcb)

    nc.finalize()
    _NC_CACHE["nc"] = nc
    return nc


def make_in_maps(coords, elem):
    """Host-side sharding + layout prep (no physics)."""
    coords = np.asarray(coords, dtype=np.float32).reshape(M, A, 3)
    elem = np.asarray(elem).reshape(M, A)
    in_maps = []
    lt = (np.arange(A)[:, None] < np.arange(A)[None, :]).astype(np.float32)
    neq = np.ones((A, NMI), np.float32)
    for i in range(A):
        for m in range(M4):
            neq[i, m * A + i] = 0.0
    rhoR = np.broadcast_to(SHF_R, (A, 16))
    rho2 = np.broadcast_to(2.0 * SHF_A, (128, 4))
    cczC = np.broadcast_to((0.475 * np.cos(SHF_Z)).astype(np.float32),
                           (128, 8))
    sszC = np.broadcast_to((0.5 * np.sin(SHF_Z)).astype(np.float32), (128, 8))
    # fp16 blob pieces (input independent)
    io_nmi = np.ascontiguousarray(np.broadcast_to(
        np.arange(NPAD, dtype=np.float16)[:, None], (NPAD, NMI)
    )).reshape(1, NPAD * NMI)
    io_nmi = np.broadcast_to(io_nmi, (A, NPAD * NMI))
    bprime = np.arange(20, dtype=np.float16)
    io20 = np.zeros((128, 20 * QP), np.float16)
    io20[:64] = np.repeat(bprime, QP)
    io20[64:] = np.repeat(bprime - 10.0, QP)
    e1 = (N1[None, :] == np.arange(NPAD)[:, None]).astype(np.float16)
    e2 = (N2[None, :] == np.arange(NPAD)[:, None]).astype(np.float16)
    for c in range(N_CORES):
        cm = coords[c * M4:(c + 1) * M4]          # [M4, A, 3]
        em = elem[c * M4:(c + 1) * M4]            # [M4, A]
        xj = cm.transpose(1, 0, 2).reshape(A, M4 * 3)
        xi = np.broadcast_to(cm.reshape(1, NMI * 3), (A, NMI * 3))
        sj = em.transpose(1, 0).astype(np.float32)
        ohmf = np.zeros((A, 16), np.float16)
        for m in range(M4):
            for s in range(4):
                ohmf[:, m * 4 + s] = (em[m] == s)
        vals = {"xj": xj, "xi": xi, "sj": sj, "lt": lt, "neq": neq,
                "rhoR": rhoR, "rho2": rho2, "ccz": cczC, "ssz": sszC}
        blob = np.zeros((128, CST_COLS), np.float32)
        o = 0
        for nm, wd in CST_LAYOUT_A + CST_LAYOUT_F:
            arr = vals[nm]
            blob[:arr.shape[0], o:o + wd] = arr
            o += wd
        valsh = {"io_nmi": io_nmi, "io20": io20, "e1": e1, "e2": e2,
                 "ohmf": ohmf}
        blobh = np.zeros((128, CSTH_COLS), np.float16)
        o = 0
        for nm, wd in CSTH_LAYOUT:
            arr = valsh[nm]
            blobh[:arr.shape[0], o:o + wd] = arr
            o += wd
        in_maps.append({"cst": blob, "csth": blobh})
    return in_maps


def kernel(elem_idxs, coords):
    elem_idxs = np.asarray(elem_idxs)
    coords = np.asarray(coords)
    nc = build_nc()
    in_maps = make_in_maps(coords, elem_idxs)
    res = run_bass_kernel_spmd(nc, in_maps, core_ids=list(range(N_CORES)))
    outs = [res.results[c]["out"] for c in range(N_CORES)]
    return np.concatenate(outs, axis=0).astype(np.float32)


# revision 18
# speedup vs baseline: 1.3276x; 1.0986x over previous
"""AEV (ANI-1x) computer on 8 Trainium2 NeuronCores.

Data-parallel over molecules: each core computes 4 of the 32 molecules.
v2: activation-table grouping (sqrt/sin/ln/exp phases), 3-op azv build,
fp16 radial matmuls scheduled early, 2-center-stacked bin-reduce (80
matmuls) with PSUM-direct output DMAs, 2x-mode one-hot builds.
"""
import sys
import numpy as np

sys.path.insert(0, "/opt/trn_rl_repo")

from concourse import bacc, bass, mybir, tile  # noqa: E402
from concourse.tile import add_dep_helper  # noqa: E402
from concourse.bass_utils import run_bass_kernel_spmd  # noqa: E402

# ---- problem constants ----
M, A = 32, 40
N_CORES = 8
M4 = M // N_CORES           # molecules per core
NMI = M4 * A                # (m, i) centers per core = 160
RCR, RCA = 5.2, 3.5
ETA_R, ETA_A, ZETA = 16.0, 8.0, 32.0
SHF_R = np.linspace(0.9, RCR, 17, dtype=np.float32)[:-1]    # 16
SHF_A = np.array([0.9, 1.55, 2.2, 2.85], dtype=np.float32)  # 4
SHF_Z = (np.pi / 16.0) * (2.0 * np.arange(8, dtype=np.float32) + 1.0)
NMAX = 11                   # max angular neighbors supported (data max = 10)
NPAD = 16                   # padded neighbor slots for the gather matmul
T = NMAX * (NMAX - 1) // 2  # 55 pair-slots per center
N1 = np.array([a for a in range(NMAX) for b in range(a + 1, NMAX)])
N2 = np.array([b for a in range(NMAX) for b in range(a + 1, NMAX)])
NARR = 8                    # gathered arrays: ux,uy,uz,d,spec,valid,0,0
QP = NMI // 2               # center-pairs in packed slot layout = 80
LN2 = float(np.log(2.0))
FP32 = mybir.dt.float32
FP16 = mybir.dt.float16
Alu = mybir.AluOpType
Act = mybir.ActivationFunctionType

# fp32 constant blob layout (rows x cols); A-rows section then 128-rows
CST_LAYOUT_A = [("xj", M4 * 3), ("xi", NMI * 3), ("sj", M4), ("lt", A),
                ("neq", NMI)]
CST_LAYOUT_F = [("rho2", 4), ("ccz", 8), ("ssz", 8), ("neq3", 54),
                ("rhoRB", 16)]
CST_A_COLS = sum(w for _, w in CST_LAYOUT_A)
CST_COLS = CST_A_COLS + sum(w for _, w in CST_LAYOUT_F)

# fp16 constant blob layout
CSTH_LAYOUT = [("io_nmi", NPAD * NMI), ("io20", 20 * QP), ("e1", T),
               ("e2", T), ("ohm3", 32)]
# radial slab decomposition: (group, col0, width, molecule)
RAD_SLABS = [(0, 0, 28, 0), (0, 28, 12, 0), (0, 40, 14, 1), (1, 0, 26, 1),
             (1, 26, 28, 2), (2, 0, 12, 2), (2, 12, 28, 3), (2, 40, 12, 3)]
CSTH_COLS = sum(w for _, w in CSTH_LAYOUT)

_NC_CACHE = {}


def build_nc():
    if "nc" in _NC_CACHE:
        return _NC_CACHE["nc"]
    from contextlib import ExitStack
    nc = bacc.Bacc()
    cst_e = nc.declare_dram_parameter("cst", [128, CST_COLS], FP32,
                                      isOutput=False)
    csth_e = nc.declare_dram_parameter("csth", [128, CSTH_COLS], FP16,
                                       isOutput=False)
    out_e = nc.declare_dram_parameter("out", [M4, A, 384], FP32, isOutput=True)

    with tile.TileContext(nc) as tc, ExitStack() as es:
        pool = es.enter_context(tc.tile_pool(name="sb", bufs=1))
        psum = es.enter_context(tc.tile_pool(name="ps", bufs=1, space="PSUM"))
        v = nc.vector
        sc = nc.scalar
        gp = nc.gpsimd

        # activation bias constants
        for cval in (1e-12, float(np.pi / 2), 1.0, 0.5 + 1e-7, LN2, 1e-35):
            cpk = (FP32, cval)
            if cpk not in nc.const_aps.aps:
                ct = pool.tile([128, 1], FP32, name=f"cst{len(nc.const_aps.aps)}")
                v.memset(ct[:, :], cval)
                nc.const_aps.aps[cpk] = ct

        # ---- load input/constant blobs across queues ----
        cst = pool.tile([128, CST_COLS], FP32)
        HC = CST_A_COLS // 2
        nc.sync.dma_start(out=cst[:A, :HC], in_=cst_e[:A, :HC])
        nc.scalar.dma_start(out=cst[:A, HC:CST_A_COLS],
                            in_=cst_e[:A, HC:CST_A_COLS])
        nc.scalar.dma_start(out=cst[:, CST_A_COLS:], in_=cst_e[:, CST_A_COLS:])
        csth = pool.tile([128, CSTH_COLS], FP16)
        HH = NPAD * NMI  # io_nmi section (rows :A)
        nc.sync.dma_start(out=csth[:A, :HH], in_=csth_e[:A, :HH])
        nc.gpsimd.dma_start(out=csth[:, HH:], in_=csth_e[:, HH:])

        off = {}
        o = 0
        for nm, wd in CST_LAYOUT_A + CST_LAYOUT_F:
            off[nm] = o
            o += wd
        offh = {}
        o = 0
        for nm, wd in CSTH_LAYOUT:
            offh[nm] = o
            o += wd

        def cv(nm, rows, wd):
            return cst[0:rows, off[nm]:off[nm] + wd]

        def cvh(nm, rows, wd):
            return csth[0:rows, offh[nm]:offh[nm] + wd]

        xj = cv("xj", A, M4 * 3)
        xi = cv("xi", A, NMI * 3)
        sj = cv("sj", A, M4)
        lt = cv("lt", A, A)
        neq = cv("neq", A, NMI)
        rho2 = cv("rho2", 128, 4)
        neq3 = cv("neq3", 120, 54)
        rhoRB = cv("rhoRB", 120, 16)
        cczC = cv("ccz", 128, 8)
        sszC = cv("ssz", 128, 8)
        io_nmi = cvh("io_nmi", A, NPAD * NMI)     # [A, (n, mi)] = n
        io20b = cvh("io20", 128, 20 * QP)         # [128, (b', q)] = b'-10*(p>=64)
        e1c = cvh("e1", NPAD, T)
        e2c = cvh("e2", NPAD, T)
        ohm3 = cvh("ohm3", 120, 32)               # [(g,j), (slab, s)] fp16

        # activation-table group bookkeeping: all T-group acts depend on all
        # S-group acts, etc.  Groups: S=sqrt, T=sin, L=ln, X=exp.
        act_groups = {"S": [], "T": [], "L": [], "X": []}
        order = ["S", "T", "L", "X"]

        def act(group, *args, **kw):
            ins = sc.activation(*args, **kw)
            gi = order.index(group)
            for gprev in order[:gi]:
                for prev in act_groups[gprev]:
                    add_dep_helper(ins.ins, prev.ins, sync=False,
                                   reason="act table order")
            act_groups[group].append(ins)
            return ins

        # ================= pair stage: [40 j, 160 (m,i)] =================
        D8 = pool.tile([A, NMI * NARR], FP16)   # gather-matmul rhs, a-major
        gp.memset(D8[:, 6 * NMI:8 * NMI], 0.0)

        def d8slot(k):
            return D8[:, k * NMI:(k + 1) * NMI]

        dx = [pool.tile([A, NMI], FP32, name=f"dx{c}", tag=f"dx{c}")
              for c in range(3)]
        for c in range(3):
            in_j = bass.AP(tensor=xj.tensor, offset=xj.offset + c,
                           ap=[xj.ap[0], [3, M4], [0, A]])
            in_i = bass.AP(tensor=xi.tensor, offset=xi.offset + c,
                           ap=[xi.ap[0], [3 * A, M4], [3, A]])
            v.tensor_tensor(dx[c][:, :].rearrange("p (m i) -> p m i", m=M4),
                            in_j, in_i, op=Alu.subtract)
        dsq = pool.tile([A, NMI], FP32)
        t0 = pool.tile([A, NMI], FP32, tag="t0")
        t1 = pool.tile([A, NMI], FP32, tag="t1")
        v.tensor_tensor(t0[:, :], dx[0][:, :], dx[0][:, :], op=Alu.mult)
        v.tensor_tensor(t1[:, :], dx[1][:, :], dx[1][:, :], op=Alu.mult)
        v.tensor_tensor(t0[:, :], t0[:, :], t1[:, :], op=Alu.add)
        v.tensor_tensor(t1[:, :], dx[2][:, :], dx[2][:, :], op=Alu.mult)
        v.tensor_tensor(dsq[:, :], t0[:, :], t1[:, :], op=Alu.add)
        # masks
        maskA = pool.tile([A, NMI], FP32)
        v.tensor_scalar(t0[:, :], dsq[:, :], RCA * RCA, None, op0=Alu.is_lt)
        v.tensor_tensor(maskA[:, :], t0[:, :], neq, op=Alu.mult)
        v.tensor_copy(d8slot(5), maskA[:, :])
        # d (sqrt, S group); raw dx into D8 (normalized later in slot space)
        dpair = pool.tile([A, NMI], FP32)
        act("S", dpair[:, :], dsq[:, :], Act.Sqrt, bias=1e-12, scale=1.0)
        for c in range(3):
            v.tensor_copy(d8slot(c), dx[c][:, :])
        v.tensor_copy(d8slot(3), dpair[:, :])
        # species of j replicated along i
        in_s = bass.AP(tensor=sj.tensor, offset=sj.offset,
                       ap=[sj.ap[0], [1, M4], [0, A]])
        v.tensor_copy(d8slot(4).rearrange("p (m i) -> p m i", m=M4), in_s)

        # ================= neighbor ranks via PE =================
        ps_rank = psum.tile([A, NMI], FP32, tag="ps", bufs=2)
        nc.tensor.matmul(ps_rank[:, :], lt, maskA[:, :], start=True, stop=True)
        rankp = pool.tile([A, NMI], FP32)
        rankp16 = pool.tile([A, NMI], FP16)
        # valid j -> rank (0..10); invalid -> rank - 1000
        v.scalar_tensor_tensor(rankp[:, :], maskA[:, :], 1000.0, ps_rank[:, :],
                               op0=Alu.mult, op1=Alu.add)
        v.tensor_scalar(rankp16[:, :], rankp[:, :], 1000.0, None,
                        op0=Alu.subtract)
        # one-hot Nsel [40, (n, mi)] fp16 n-major (2x DVE mode: all operands
        # fp16 with packed inner mi)
        Nsel = pool.tile([A, NPAD * NMI], FP16)
        v.tensor_tensor(
            Nsel[:, :].rearrange("p (n mi) -> p n mi", n=NPAD),
            rankp16[:, :].unsqueeze(1).broadcast_to((A, NPAD, NMI)),
            io_nmi.rearrange("p (n mi) -> p n mi", n=NPAD),
            op=Alu.is_equal)

        # ---- radial early work, [120 = (g, j), 54] layout ----
        dsq3 = pool.tile([120, 54], FP32)
        for g in range(3):
            wg = 54 if g < 2 else 52
            (nc.sync, nc.scalar, nc.gpsimd)[g].dma_start(
                out=dsq3[40 * g:40 * (g + 1), :wg],
                in_=dsq[:, 54 * g:54 * g + wg])
        maskR3 = pool.tile([120, 54], FP32)
        v.tensor_scalar(maskR3[:, :], dsq3[:, :], RCR * RCR, None,
                        op0=Alu.is_lt)
        v.tensor_tensor(maskR3[:, :], maskR3[:, :], neq3, op=Alu.mult)
        dp3 = pool.tile([120, 54], FP32)
        act("S", dp3[:, :], dsq3[:, :], Act.Sqrt, bias=1e-12, scale=1.0)
        dcl3 = pool.tile([120, 54], FP32)
        v.tensor_scalar(dcl3[:, :], dp3[:, :], RCR, None, op0=Alu.min)
        RT3 = pool.tile([120, 54 * 16], FP32)
        v.tensor_tensor(
            RT3[:, :].rearrange("p (c r) -> p c r", r=16),
            dp3[:, :].unsqueeze(2).broadcast_to((120, 54, 16)),
            rhoRB.unsqueeze(1).broadcast_to((120, 54, 16)),
            op=Alu.subtract)
        sc.activation(RT3[:, :], RT3[:, :], Act.Square, scale=4.0)

        # ====== per-center gather matmuls, two halves so the first
        # gsb copy + expansion overlap the second half's gathers ======
        gsb = pool.tile([NPAD, NMI * NARR], FP16)
        GH = NMI // 2
        for gh in range(2):
            ps_gath = psum.tile([NPAD, GH * NARR], FP32, tag="ps", bufs=2,
                                name=f"ps_gath{gh}")
            for ci in range(GH):
                cidx = gh * GH + ci
                lhs = bass.AP(tensor=Nsel.tensor,
                              offset=Nsel[:, :].offset + cidx,
                              ap=[Nsel[:, :].ap[0], [NMI, NPAD]])
                rhs = bass.AP(tensor=D8.tensor,
                              offset=D8[:, :].offset + cidx,
                              ap=[D8[:, :].ap[0], [NMI, NARR]])
                nc.tensor.matmul(
                    ps_gath[:, ci * NARR:(ci + 1) * NARR],
                    lhs, rhs,
                    start=True, stop=True)
            eng = sc.copy if gh == 0 else v.tensor_copy
            eng(gsb[:, gh * GH * NARR:(gh + 1) * GH * NARR], ps_gath[:, :])

        # ========= slot expansion via constant one-hot matmuls =========
        # X1/X2 sbuf: [128 = (cpar*64 + t), (q, arr)], center = 2q + cpar
        X1 = pool.tile([128, QP * NARR], FP32)
        X2 = pool.tile([128, QP * NARR], FP32)
        # zero (covers the unused rows 55..63 / 119..127 of each block)
        gp.memset(X1[:, :], 0.0)
        gp.memset(X2[:, :], 0.0)
        QWID = NMI * NARR // 4
        xcopy_engs = [sc.copy, v.tensor_copy]
        xci = 0
        for side, ec, X in ((0, e1c, X1), (1, e2c, X2)):
            for hh in range(4):
                ps_x = psum.tile([T, QWID], FP32, tag="psx", bufs=2,
                                 name=f"ps_x{side}{hh}")
                nc.tensor.matmul(ps_x[:, :], ec,
                                 gsb[:, hh * QWID:(hh + 1) * QWID],
                                 start=True, stop=True)
                # quarter hh covers centers [hh*40, +40) -> q in [hh*20, +20)
                for cpar in range(2):
                    srcx = bass.AP(tensor=ps_x.tensor,
                                   offset=ps_x.offset + cpar * NARR,
                                   ap=[ps_x.ap[0], [2 * NARR, QP // 4],
                                       [1, NARR]])
                    dstx = X[cpar * 64:cpar * 64 + T,
                             hh * QP * NARR // 4:(hh + 1) * QP * NARR // 4]
                    xcopy_engs[xci % len(xcopy_engs)](
                        dstx.rearrange("p (q a) -> p q a", a=NARR), srcx)
                    xci += 1

        def xs(X, k):
            return X[:, :].rearrange("p (q a) -> p q a", a=NARR)[:, :, k]

        # ================= slot math: [128, 80] =================
        P2 = 128
        cdot = pool.tile([P2, QP], FP32)
        ta = pool.tile([P2, QP], FP32, tag="ta")
        tb = pool.tile([P2, QP], FP32, tag="tb")
        v.tensor_tensor(cdot[:, :], xs(X1, 0), xs(X2, 0), op=Alu.mult)
        v.tensor_tensor(ta[:, :], xs(X1, 1), xs(X2, 1), op=Alu.mult)
        v.tensor_tensor(cdot[:, :], cdot[:, :], ta[:, :], op=Alu.add)
        v.tensor_tensor(ta[:, :], xs(X1, 2), xs(X2, 2), op=Alu.mult)
        v.tensor_tensor(cdot[:, :], cdot[:, :], ta[:, :], op=Alu.add)
        # normalize: cdot /= d1*d2 + eps (eps keeps zeroed pad rows finite)
        v.tensor_tensor(ta[:, :], xs(X1, 3), xs(X2, 3), op=Alu.mult)
        v.tensor_scalar(ta[:, :], ta[:, :], 1e-12, None, op0=Alu.add)
        inv12 = pool.tile([P2, QP], FP32)
        v.reciprocal(inv12[:, :], ta[:, :])
        v.tensor_tensor(cdot[:, :], cdot[:, :], inv12[:, :], op=Alu.mult)
        # s = sqrt(1 - (0.95 c)^2) -- S group (before the sins)
        csq = pool.tile([P2, QP], FP32)
        sc.activation(csq[:, :], cdot[:, :], Act.Square, scale=0.95)
        svar = pool.tile([P2, QP], FP32)
        act("S", svar[:, :], csq[:, :], Act.Sqrt, bias=1.0, scale=-1.0)
        # cutoffs (trig group)
        sinr3 = pool.tile([120, 54], FP32)
        act("T", sinr3[:, :], dcl3[:, :], Act.Sin, bias=np.pi / 2,
            scale=-np.pi / RCR)
        sin1 = pool.tile([P2, QP], FP32)
        sin2 = pool.tile([P2, QP], FP32)
        act("T", sin1[:, :], xs(X1, 3), Act.Sin, bias=np.pi / 2,
            scale=-np.pi / RCA)
        act("T", sin2[:, :], xs(X2, 3), Act.Sin, bias=np.pi / 2,
            scale=-np.pi / RCA)
        # w = fc1*fc2*valid1*valid2
        w = pool.tile([P2, QP], FP32)
        v.tensor_scalar(ta[:, :], sin1[:, :], 0.5, 0.5, op0=Alu.mult,
                        op1=Alu.add)
        v.tensor_scalar(tb[:, :], sin2[:, :], 0.5, 0.5, op0=Alu.mult,
                        op1=Alu.add)
        v.tensor_tensor(w[:, :], ta[:, :], tb[:, :], op=Alu.mult)
        v.tensor_tensor(ta[:, :], xs(X1, 5), xs(X2, 5), op=Alu.mult)
        v.tensor_tensor(w[:, :], w[:, :], ta[:, :], op=Alu.mult)
        # radial fc chain (vector)
        fcr3 = pool.tile([120, 54], FP32)
        v.tensor_scalar(fcr3[:, :], sinr3[:, :], 0.125, 0.125,
                        op0=Alu.mult, op1=Alu.add)
        gr3 = pool.tile([120, 54], FP32)
        v.tensor_tensor(gr3[:, :], fcr3[:, :], maskR3[:, :], op=Alu.mult)

        # f2 = 2*exp(-2(u-2rho)^2) * w   [128, (q, a)] q-major
        u = pool.tile([P2, QP], FP32)
        v.tensor_tensor(u[:, :], xs(X1, 3), xs(X2, 3), op=Alu.add)
        f2 = pool.tile([P2, QP * 4], FP32)
        v.tensor_tensor(
            f2[:, :].rearrange("p (q a) -> p q a", a=4),
            u[:, :].unsqueeze(2).broadcast_to((P2, QP, 4)),
            rho2.unsqueeze(1).broadcast_to((128, QP, 4)),
            op=Alu.subtract)
        sc.activation(f2[:, :], f2[:, :], Act.Square, scale=float(np.sqrt(2.0)))

        # azv = 0.5 + 0.475 cos(phi_z) c + 0.5 sin(phi_z) s  [128, (q, z)]
        azv = pool.tile([P2, QP * 8], FP32)
        tz = pool.tile([P2, QP * 8], FP32)
        v.tensor_tensor(
            azv[:, :].rearrange("p (q z) -> p q z", z=8),
            cdot[:, :].unsqueeze(2).broadcast_to((P2, QP, 8)),
            cczC.unsqueeze(1).broadcast_to((128, QP, 8)),
            op=Alu.mult)
        v.tensor_tensor(
            tz[:, :].rearrange("p (q z) -> p q z", z=8),
            svar[:, :].unsqueeze(2).broadcast_to((P2, QP, 8)),
            sszC.unsqueeze(1).broadcast_to((128, QP, 8)),
            op=Alu.mult)
        v.tensor_tensor(azv[:, :], azv[:, :], tz[:, :], op=Alu.add)
        # ln phase (L group): azv ln
        act("L", azv[:, :], azv[:, :], Act.Ln, bias=0.5 + 1e-7, scale=1.0)
        # exp phase (X group): azvb = f1 in fp16, f2 exp, radial exp
        azvb = pool.tile([P2, QP * 8], FP16)
        act("X", azvb[:, :], azv[:, :], Act.Exp, scale=float(ZETA))
        act("X", f2[:, :], f2[:, :], Act.Exp, bias=LN2, scale=-1.0)
        RTe3 = pool.tile([120, 54 * 16], FP32)
        act("X", RTe3[:, :], RT3[:, :], Act.Exp, scale=-1.0)
        RTf3 = pool.tile([120, 54 * 16], FP16)
        v.tensor_tensor(
            RTf3[:, :].rearrange("p (c r) -> p c r", r=16),
            RTe3[:, :].rearrange("p (c r) -> p c r", r=16),
            gr3[:, :].unsqueeze(2).broadcast_to((120, 54, 16)),
            op=Alu.mult)

        # ---- radial matmuls: 8 slabs, [120 contract, 4 species] ----
        rad_sb = pool.tile([4, NMI * 16], FP32)
        rad_engs = [sc.copy, v.tensor_copy]
        mol_ready = {}
        for si, (g, c0, wd, mol) in enumerate(RAD_SLABS):
            ps_rad = psum.tile([4, wd * 16], FP32, tag="psx", bufs=2,
                               name=f"ps_rad{si}")
            nc.tensor.matmul(
                ps_rad[:, :],
                ohm3[:, si * 4:(si + 1) * 4],
                RTf3[:, c0 * 16:(c0 + wd) * 16],
                start=True, stop=True)
            cg0 = g * 54 + c0
            rad_engs[si % 2](rad_sb[:, cg0 * 16:(cg0 + wd) * 16],
                             ps_rad[:, :])
            mol_ready[mol] = mol_ready.get(mol, 0) + 1
            if mol_ready[mol] == 2:
                dst_rad = bass.AP(tensor=out_e[:, :, :].tensor,
                                  offset=mol * 40 * 384,
                                  ap=[[16, 4], [384, 40], [1, 16]])
                base = rad_sb[:, mol * 640:(mol + 1) * 640]
                src_rad = bass.AP(tensor=base.tensor, offset=base.offset,
                                  ap=[base.ap[0], [16, 40], [1, 16]])
                dma_q = (nc.sync, nc.scalar, nc.gpsimd, nc.sync)[mol]
                dma_q.dma_start(out=dst_rad, in_=src_rad)

        # f2 *= w; write fp16 for the F build
        f2b = pool.tile([P2, QP * 4], FP16)
        v.tensor_tensor(
            f2b[:, :].rearrange("p (q a) -> p q a", a=4),
            f2[:, :].rearrange("p (q a) -> p q a", a=4),
            w[:, :].unsqueeze(2).broadcast_to((P2, QP, 4)),
            op=Alu.mult)

        # F [128, (q, a, z)] fp16 = f2b (q,a) x azvb (q,z)
        F = pool.tile([P2, QP * 32], FP16)
        HF = QP // 2
        for fh in range(2):
            ap_f2 = bass.AP(tensor=f2b.tensor,
                            offset=f2b[:, :].offset + fh * HF * 4,
                            ap=[f2b[:, :].ap[0], [4, HF], [1, 4], [0, 8]])
            ap_f1 = bass.AP(tensor=azvb.tensor,
                            offset=azvb[:, :].offset + fh * HF * 8,
                            ap=[azvb[:, :].ap[0], [8, HF], [0, 4], [1, 8]])
            fdst = F[:, fh * HF * 32:(fh + 1) * HF * 32]
            v.tensor_tensor(fdst.rearrange("p (q a z) -> p q a z", a=4, z=8),
                            ap_f2, ap_f1, op=Alu.mult)

        # W2 one-hot [128, (b', q)] fp16, b' = bin + 10*cpar via io20b
        pv = pool.tile([P2, QP], FP32)
        tc2 = pool.tile([P2, QP], FP32, tag="tc")
        pv16 = pool.tile([P2, QP], FP16)
        v.tensor_tensor(tc2[:, :], xs(X1, 4), xs(X2, 4), op=Alu.max)
        v.scalar_tensor_tensor(pv[:, :], xs(X1, 4), 0.0, tc2[:, :],
                               op0=Alu.mult, op1=Alu.add)
        v.tensor_tensor(tc2[:, :], xs(X1, 4), xs(X2, 4), op=Alu.min)
        v.scalar_tensor_tensor(pv[:, :], tc2[:, :], 3.5, pv[:, :],
                               op0=Alu.mult, op1=Alu.add)
        v.tensor_tensor(tc2[:, :], tc2[:, :], tc2[:, :], op=Alu.mult)
        v.scalar_tensor_tensor(pv16[:, :], tc2[:, :], -0.5, pv[:, :],
                               op0=Alu.mult, op1=Alu.add)
        W2 = pool.tile([P2, 20 * QP], FP16)
        v.tensor_tensor(
            W2[:, :].rearrange("p (b q) -> p b q", b=20),
            pv16[:, :].unsqueeze(1).broadcast_to((P2, 20, QP)),
            io20b.rearrange("p (b q) -> p b q", b=20),
            op=Alu.is_equal)

        # ================= angular bin-reduce =================
        # 2-center-stacked: one matmul per q -> psum [20, 32]; DMA rounds
        # straight from PSUM to HBM (2 DMAs per round, one per parity).
        RQ = 16                      # q per round
        out_sb = pool.tile([20, QP * 32], FP32)
        dma_qs = [nc.sync, nc.scalar, nc.gpsimd]
        cp_engs = [v.tensor_copy, sc.copy]
        for rnd in range(QP // RQ):
            ps_bin = psum.tile([20, RQ * 32], FP32, tag="psb", bufs=2,
                               name=f"ps_bin{rnd}")
            for qi in range(RQ):
                q = rnd * RQ + qi
                lhs = bass.AP(tensor=W2.tensor,
                              offset=W2[:, :].offset + q,
                              ap=[W2[:, :].ap[0], [QP, 20]])
                nc.tensor.matmul(
                    ps_bin[:, qi * 32:(qi + 1) * 32],
                    lhs,
                    F[:, q * 32:(q + 1) * 32],
                    start=True, stop=True)
            sl = out_sb[:, rnd * RQ * 32:(rnd + 1) * RQ * 32]
            cp_engs[rnd % len(cp_engs)](sl, ps_bin[:, :])
            for cpar in range(2):
                dstb = bass.AP(tensor=out_e[:, :, :].tensor,
                               offset=(2 * RQ * rnd + cpar) * 384 + 64,
                               ap=[[32, 10], [2 * 384, RQ], [1, 32]])
                sb_sub = out_sb[cpar * 10:cpar * 10 + 10,
                                rnd * RQ * 32:(rnd + 1) * RQ * 32]
                srcb = bass.AP(tensor=sb_sub.tensor, offset=sb_sub.offset,
                               ap=[sb_sub.ap[0], [32, RQ], [1, 32]])
                dma_qs[(rnd * 2 + cpar) % len(dma_qs)].dma_start(
                    out=dstb, in_=srcb)

    nc.finalize()
    _NC_CACHE["nc"] = nc
    return nc


def make_in_maps(coords, elem):
    """Host-side sharding + layout prep (no physics)."""
    coords = np.asarray(coords, dtype=np.float32).reshape(M, A, 3)
    elem = np.asarray(elem).reshape(M, A)
    in_maps = []
    lt = (np.arange(A)[:, None] < np.arange(A)[None, :]).astype(np.float32)
    neq = np.ones((A, NMI), np.float32)
    for i in range(A):
        for m in range(M4):
            neq[i, m * A + i] = 0.0
    rho2 = np.broadcast_to(2.0 * SHF_A, (128, 4))
    rhoRB = np.broadcast_to(SHF_R, (120, 16))
    # neq3[(g,j), c_local]: 1 unless j == atom index of center, or pad col
    neq3 = np.ones((120, 54), np.float32)
    for g in range(3):
        for cl in range(54):
            cg = g * 54 + cl
            if cg >= NMI:
                neq3[40 * g:40 * (g + 1), cl] = 0.0
            else:
                neq3[40 * g + (cg % A), cl] = 0.0
    cczC = np.broadcast_to((0.475 * np.cos(SHF_Z)).astype(np.float32),
                           (128, 8))
    sszC = np.broadcast_to((0.5 * np.sin(SHF_Z)).astype(np.float32), (128, 8))
    # fp16 blob pieces (input independent)
    io_nmi = np.ascontiguousarray(np.broadcast_to(
        np.arange(NPAD, dtype=np.float16)[:, None], (NPAD, NMI)
    )).reshape(1, NPAD * NMI)
    io_nmi = np.broadcast_to(io_nmi, (A, NPAD * NMI))
    bprime = np.arange(20, dtype=np.float16)
    io20 = np.zeros((128, 20 * QP), np.float16)
    io20[:64] = np.repeat(bprime, QP)
    io20[64:] = np.repeat(bprime - 10.0, QP)
    e1 = (N1[None, :] == np.arange(NPAD)[:, None]).astype(np.float16)
    e2 = (N2[None, :] == np.arange(NPAD)[:, None]).astype(np.float16)
    for c in range(N_CORES):
        cm = coords[c * M4:(c + 1) * M4]          # [M4, A, 3]
        em = elem[c * M4:(c + 1) * M4]            # [M4, A]
        xj = cm.transpose(1, 0, 2).reshape(A, M4 * 3)
        xi = np.broadcast_to(cm.reshape(1, NMI * 3), (A, NMI * 3))
        sj = em.transpose(1, 0).astype(np.float32)
        ohm3 = np.zeros((120, 32), np.float16)
        for si, (g, c0, wd, mol) in enumerate(RAD_SLABS):
            for s in range(4):
                ohm3[40 * g:40 * (g + 1), si * 4 + s] = (em[mol] == s)
        vals = {"xj": xj, "xi": xi, "sj": sj, "lt": lt, "neq": neq,
                "rho2": rho2, "ccz": cczC, "ssz": sszC, "neq3": neq3,
                "rhoRB": rhoRB}
        blob = np.zeros((128, CST_COLS), np.float32)
        o = 0
        for nm, wd in CST_LAYOUT_A + CST_LAYOUT_F:
            arr = vals[nm]
            blob[:arr.shape[0], o:o + wd] = arr
            o += wd
        valsh = {"io_nmi": io_nmi, "io20": io20, "e1": e1, "e2": e2,
                 "ohm3": ohm3}
        blobh = np.zeros((128, CSTH_COLS), np.float16)
        o = 0
        for nm, wd in CSTH_LAYOUT:
            arr = valsh[nm]
            blobh[:arr.shape[0], o:o + wd] = arr
            o += wd
        in_maps.append({"cst": blob, "csth": blobh})
    return in_maps


def kernel(elem_idxs, coords):
    elem_idxs = np.asarray(elem_idxs)
    coords = np.asarray(coords)
    nc = build_nc()
    in_maps = make_in_maps(coords, elem_idxs)
    res = run_bass_kernel_spmd(nc, in_maps, core_ids=list(range(N_CORES)))
    outs = [res.results[c]["out"] for c in range(N_CORES)]
    return np.concatenate(outs, axis=0).astype(np.float32)
